# revision 42
# baseline (speedup 1.0000x reference)
"""Multi-head attention (B=2, S=2048, D=1024, H=16) on 8 Trainium2 cores.

Sharding: core c handles batch c//4 and heads 4*(c%4)..4*(c%4)+4 (a
256-channel slice of the QKV projections and of the Wo contraction).
Each core computes a bf16 output-projection partial over its 256
channels; the host sums the 4 partials per batch in fp32 and adds
bo_eff = bo + bv @ Wo.T (the value bias folds out of attention because
softmax rows sum to 1).

All matmul operands are bf16 (psum accumulation stays fp32); inputs are
host-prepacked to partition-major [128, ...] so every DMA is a cheap 2D
copy, and each qt/kt/vt stream loads with a single 4MB DMA on its own
queue (the projections contract over all of D, so nothing could start
before the full stream lands).

Layouts:
  qT/kT [oc=256, s]   two [128, 2048] tiles (head-pair channels)
  v     [s, oc]       per k-block [128, 4*65] with a ones column per
                      head -> MM2's row 64 yields softmax denominators
Attention runs as 128 steps (pt-outer: head-pair x q-quarter x k-block):
  MM2 batch of the previous k-block (c=128 MMs never directly follow
  the just-issued c=64 pair), then
  MM1: two c=64 matmuls on alternating PE row groups (base 0/64) --
       alternating-base c=64 pairs execute concurrently (~2 cols/cyc)
  exp: one ScalarE activation [128,1024] psum->bf16, scale=1/8 fused
Only a minimal pre-phase (K[ob0], Q[ob0,sc0], V) runs serially; the
remaining 11 projection chains and the per-q-quarter output projections
are injected two micro-ops per step into the attention's TensorMatrix
slack under the ScalarE-bound exp stream. PSUM: chain ring (2 banks) +
score ring (4) + po accumulators (2). Injected evicts are scheduled at
least a full 16-step segment before their consumers (tighter write->read
spacing showed scheduler hazards on HW).
"""

import sys

if "/opt/trn_rl_repo" not in sys.path:
    sys.path.insert(0, "/opt/trn_rl_repo")

import numpy as np

B, S, D = 2, 2048, 1024
HPC = 4            # heads per core
OC = HPC * 64      # projection output channels per core
NCORES = 8
KB = 16            # k-blocks of 128
VSTRIDE = HPC * 65 # v storage per k-block: 4 heads x (64 v + 1 ones)

_CACHE = {}


def _build(debug=False):
    import concourse.bacc as bacc
    import concourse.mybir as mybir
    from concourse.tile import TileContext

    F32 = mybir.dt.float32
    BF16 = mybir.dt.bfloat16
    AF = mybir.ActivationFunctionType

    # all inputs host-prepacked to partition-major [128, ...] so every DMA
    # is a cheap 2D copy (8-way rearrange DMAs cost ~14us descriptor gen)
    nc = bacc.Bacc(None, target_bir_lowering=False)
    qt_in = nc.dram_tensor("qt", [128, 8 * S], BF16, kind="ExternalInput")
    kt_in = nc.dram_tensor("kt", [128, 8 * S], BF16, kind="ExternalInput")
    vt_in = nc.dram_tensor("vt", [128, 8 * S], BF16, kind="ExternalInput")
    wq_in = nc.dram_tensor("wq", [128, 8 * OC], BF16, kind="ExternalInput")
    wk_in = nc.dram_tensor("wk", [128, 8 * OC], BF16, kind="ExternalInput")
    wv_in = nc.dram_tensor("wv", [128, 8 * OC], BF16, kind="ExternalInput")
    wo_in = nc.dram_tensor("wo", [OC, D], BF16, kind="ExternalInput")
    bq_in = nc.dram_tensor("bq", [128, 2], F32, kind="ExternalInput")
    bk_in = nc.dram_tensor("bk", [128, 2], F32, kind="ExternalInput")
    out = nc.dram_tensor("out", [S, D], BF16, kind="ExternalOutput")
    dbg = {}
    if debug:
        for n, w in (("kT0", S), ("kT1", S), ("qT0", S), ("qT1", S),
                     ("ao0", S), ("ao1", S), ("vsb", KB * VSTRIDE)):
            dbg[n] = nc.dram_tensor("dbg_" + n, [128, w], F32,
                                    kind="ExternalOutput")

    with TileContext(nc) as tc:
        with tc.tile_pool(name="wpool", bufs=1) as wp, \
             tc.tile_pool(name="data", bufs=1) as dp:
            wq_sb = wp.tile([128, 8 * OC], BF16, name="wq_sb")
            wk_sb = wp.tile([128, 8 * OC], BF16, name="wk_sb")
            wv_sb = wp.tile([128, 8 * OC], BF16, name="wv_sb")
            wo_sb = [wp.tile([128, D], BF16, name=f"wo{p}") for p in range(2)]
            bq_sb = wp.tile([128, 2], F32, name="bq_sb")
            bk_sb = wp.tile([128, 2], F32, name="bk_sb")
            # weights on the gpsimd queue ahead of vt; wo (needed last)
            # after vt; sync + scalar queues carry the kt/qt streams
            nc.gpsimd.dma_start(wk_sb[:], wk_in[:, :])
            nc.gpsimd.dma_start(bk_sb[:], bk_in[:, :])
            nc.gpsimd.dma_start(wq_sb[:], wq_in[:, :])
            nc.gpsimd.dma_start(bq_sb[:], bq_in[:, :])
            nc.gpsimd.dma_start(wv_sb[:], wv_in[:, :])

            qT = [dp.tile([128, S], BF16, name=f"qT{p}") for p in range(2)]
            kT = [dp.tile([128, S], BF16, name=f"kT{p}") for p in range(2)]
            v_sb = dp.tile([128, KB * VSTRIDE], BF16, name="v_sb")
            ao = [dp.tile([128, S], BF16, name=f"ao{p}") for p in range(2)]
            nc.gpsimd.memset(v_sb[:], 1.0)

            # input streams: one resident [128, 8S] tile per stream, loaded
            # by a single 4MB DMA (splits across all 16 SDMA slots for max
            # bandwidth; the projection chains contract over all of D, so
            # nothing could start before the full stream lands anyway).
            # kt (sync) and qt (scalar) transfer concurrently from t=0.
            kts = [wp.tile([128, 4 * S], BF16, name=f"xk{i}") for i in range(2)]
            vts = [wp.tile([128, 4 * S], BF16, name=f"xv{i}") for i in range(2)]
            qts = [wp.tile([128, 4 * S], BF16, name=f"xq{i}") for i in range(2)]
            for i in range(2):
                nc.sync.dma_start(kts[i][:], kt_in[:, i * 4 * S:(i + 1) * 4 * S])
            for i in range(2):
                nc.scalar.dma_start(qts[i][:], qt_in[:, i * 4 * S:(i + 1) * 4 * S])
            for i in range(2):
                nc.gpsimd.dma_start(vts[i][:], vt_in[:, i * 4 * S:(i + 1) * 4 * S])
            for p in range(2):
                nc.gpsimd.dma_start(wo_sb[p][:], wo_in[p * 128:(p + 1) * 128, :])

            def xs(ts, t, sl):  # d-tile t, free-dim slice sl of [128, S]
                return ts[t // 4][:, (t % 4) * S + sl.start: (t % 4) * S + sl.stop]

            # ---- merged PSUM scope: chain ring (2 banks) + psc ring
            # (4 banks) + po ring (2 banks) = 8. The minimal pre-phase
            # (K[ob0], Q[ob0,sc0], all V) runs serially; every other
            # projection chain and the output projections are injected
            # into the attention steps' TensorMatrix slack.
            with tc.tile_pool(name="chainp", bufs=2, space="PSUM") as chp, \
                 tc.tile_pool(name="pscp", bufs=2, space="PSUM") as pscp, \
                 tc.tile_pool(name="pop", bufs=2, space="PSUM") as pop, \
                 tc.tile_pool(name="etp", bufs=6) as etp, \
                 tc.tile_pool(name="rl", bufs=2) as rlp, \
                 tc.tile_pool(name="ostage", bufs=3) as osp:

                def kq_chain(wsb, ts, bias_sb, dstT, ob, sc):
                    """8 accumulating MMs + bias-add evict for one
                    [128 oc, 512 s] projection chain."""
                    ps = chp.tile([128, 512], F32, name=f"ch{ob}{sc}",
                                  tag="chain")
                    for t in range(8):
                        def mm(t=t, ps=ps):
                            nc.tensor.matmul(
                                ps[:],
                                wsb[:, t * OC + ob * 128: t * OC + ob * 128 + 128],
                                xs(ts, t, slice(sc * 512, sc * 512 + 512)),
                                start=(t == 0), stop=(t == 7))
                        yield mm
                    def ev(ps=ps):
                        nc.vector.tensor_scalar_add(
                            dstT[ob][:, sc * 512:(sc + 1) * 512],
                            ps[:], bias_sb[:, ob:ob + 1])
                    yield ev

                def v_unit(jj):
                    """16 MMs + strided evict for s-blocks 2jj, 2jj+1.
                    One accumulation group per tile (PSUM start zeroing
                    is region-granular on HW)."""
                    ps = chp.tile([128, 512], F32, name=f"vu{jj}", tag="chain")
                    for t in range(8):
                        for h in range(2):
                            def mm(t=t, h=h, ps=ps):
                                sbk = 2 * jj + h
                                nc.tensor.matmul(
                                    ps[:, h * 256:(h + 1) * 256],
                                    xs(vts, t, slice(sbk * 128, sbk * 128 + 128)),
                                    wv_sb[:, t * OC:(t + 1) * OC],
                                    start=(t == 0 and h == 0),
                                    stop=(t == 7 and h == 1))
                            yield mm
                    def ev(ps=ps):
                        for h in range(2):
                            sbk = 2 * jj + h
                            dst = v_sb[:, sbk * VSTRIDE:(sbk + 1) * VSTRIDE] \
                                .rearrange("p (h c) -> p h c", c=65)[:, :, 0:64]
                            src = ps[:, h * 256:(h + 1) * 256] \
                                .rearrange("p (h c) -> p h c", c=64)
                            nc.vector.tensor_copy(dst, src)
                    yield ev

                def outproj_microops(qq, tail=False):
                    """Per s-block: two [128,512] chain-pool psum tiles
                    (jc halves), 2 MMs + evict each, then the out DMA.
                    The chain pool is idle once the projection chains
                    drain, so this never blocks the psc ring."""
                    for j in range(4):
                        sb = qq * 4 + j
                        ot = osp.tile([128, 1024], BF16, name=f"ot{sb}", tag="ot")
                        for jc in range(2):
                            pj = chp.tile([128, 512], F32, name=f"pj{sb}_{jc}",
                                          tag="chain")
                            for pt in range(2):
                                def mm(pj=pj, jc=jc, pt=pt, sb=sb):
                                    nc.tensor.matmul(
                                        pj[:],
                                        ao[pt][:, sb * 128:(sb + 1) * 128],
                                        wo_sb[pt][:, jc * 512:(jc + 1) * 512],
                                        start=(pt == 0), stop=(pt == 1))
                                yield mm
                            def evict(pj=pj, jc=jc, ot=ot):
                                dst = ot[:, jc * 512:(jc + 1) * 512]
                                if tail:
                                    # drain: ScalarE idle, DVE normalizing
                                    nc.scalar.activation(dst, pj[:], AF.Copy)
                                else:
                                    nc.vector.tensor_copy(dst, pj[:])
                            yield evict
                        def dma(ot=ot, sb=sb):
                            nc.sync.dma_start(out[sb * 128:(sb + 1) * 128, :], ot[:])
                        yield dma

                # pre-phase: minimal serial prerequisites for (pt0, qq0)
                for sc in range(4):
                    for op in kq_chain(wk_sb, kts, bk_sb, kT, 0, sc):
                        op()
                for op in kq_chain(wq_sb, qts, bq_sb, qT, 0, 0):
                    op()
                for jj in range(8):
                    for op in v_unit(jj):
                        op()

                # injected work, deadline-ordered (steps of 16 per segment,
                # pt-outer: segment index = pt*4 + qq). Each chain's evict
                # must land >= a full segment before its consumer (the
                # scheduler's write->read spacing is hazard-prone when the
                # gap shrinks to a step or two).
                inject_q = [
                    kq_chain(wq_sb, qts, bq_sb, qT, 0, 1),   # seg 1
                    kq_chain(wq_sb, qts, bq_sb, qT, 0, 2),   # seg 2
                    kq_chain(wq_sb, qts, bq_sb, qT, 0, 3),   # seg 3
                    kq_chain(wk_sb, kts, bk_sb, kT, 1, 0),   # seg 4
                    kq_chain(wk_sb, kts, bk_sb, kT, 1, 1),
                    kq_chain(wk_sb, kts, bk_sb, kT, 1, 2),
                    kq_chain(wk_sb, kts, bk_sb, kT, 1, 3),
                    kq_chain(wq_sb, qts, bq_sb, qT, 1, 0),   # seg 4
                    kq_chain(wq_sb, qts, bq_sb, qT, 1, 1),   # seg 5
                    kq_chain(wq_sb, qts, bq_sb, qT, 1, 2),   # seg 6
                    kq_chain(wq_sb, qts, bq_sb, qT, 1, 3),   # seg 7
                ]

                def inject(budget=2):
                    while budget > 0 and inject_q:
                        op = next(inject_q[0], None)
                        if op is None:
                            inject_q.pop(0)
                            continue
                        op()
                        budget -= 1

                for pt in range(2):
                    for qq in range(4):
                        q0 = qq * 512
                        po = [pop.tile([65, 512], F32, name=f"po{pt}_{qq}_{hh}",
                                       tag="po") for hh in range(2)]
                        prev_et = None
                        for kb in range(KB + 1):
                            if prev_et is not None:
                                # MM2s before MM1s so c=128 MMs never
                                # directly follow the c=64 MM1 pair
                                pkb = kb - 1
                                for hh in range(2):
                                    h = 2 * pt + hh
                                    va = v_sb[:, pkb * VSTRIDE + h * 65:
                                              pkb * VSTRIDE + h * 65 + 65]
                                    nc.tensor.matmul(
                                        po[hh][:], va,
                                        prev_et[:, hh * 512:(hh + 1) * 512],
                                        start=(pkb == 0), stop=(pkb == KB - 1))
                                inject()
                            if kb < KB:
                                psc = pscp.tile([128, 1024], F32,
                                                name=f"psc{pt}_{qq}_{kb}", tag="psc")
                                for hh in range(2):
                                    bp = hh * 64
                                    nc.tensor.matmul(
                                        psc[:, hh * 512:(hh + 1) * 512],
                                        kT[pt][bp:bp + 64, kb * 128:(kb + 1) * 128],
                                        qT[pt][bp:bp + 64, q0:q0 + 512],
                                        start=True, stop=True)
                                et = etp.tile([128, 1024], BF16,
                                              name=f"et{pt}_{qq}_{kb}", tag="et")
                                nc.scalar.activation(et[:], psc[:], AF.Exp,
                                                     scale=0.125)
                                prev_et = et
                            else:
                                prev_et = None
                        # normalize by the ones-column row sums (psum row 64)
                        rl1s = []
                        for hh in range(2):
                            # stage row 64 through SBUF: the custom-DVE recip
                            # mis-addresses PSUM reads at partition base != 0
                            rl0 = rlp.tile([1, 512], F32, name=f"rl0{pt}{qq}{hh}",
                                           tag=f"rl0{hh}")
                            nc.vector.tensor_copy(rl0[:], po[hh][64:65, :])
                            rl1 = rlp.tile([1, 512], F32, name=f"rl1{pt}{qq}{hh}",
                                           tag=f"rl1{hh}")
                            nc.vector.reciprocal_approx_fast(rl1[:], rl0[:])
                            rl1s.append(rl1)
                        rcps = []
                        for hh in range(2):
                            rcp = rlp.tile([64, 512], F32, name=f"rcp{pt}{qq}{hh}",
                                           tag=f"rcp{hh}")
                            nc.gpsimd.partition_broadcast(rcp[:], rl1s[hh][:])
                            rcps.append(rcp)
                        nc.vector.tensor_mul(ao[pt][0:64, q0:q0 + 512],
                                             po[0][0:64, :], rcps[0][:])
                        tmp = rlp.tile([64, 512], BF16, name=f"tm{pt}{qq}", tag="tm")
                        nc.vector.tensor_mul(tmp[:], po[1][0:64, :], rcps[1][:])
                        nc.gpsimd.dma_start(ao[pt][64:128, q0:q0 + 512], tmp[:])
                        if pt == 1:
                            inject_q.append(
                                outproj_microops(qq, tail=(qq == 3)))
                # drain remaining injected work
                while inject_q:
                    op = next(inject_q[0], None)
                    if op is None:
                        inject_q.pop(0)
                    else:
                        op()

            if debug:
                with tc.tile_pool(name="dbgp", bufs=2) as dbp:
                    for n, src in (("kT0", kT[0]), ("kT1", kT[1]),
                                   ("qT0", qT[0]), ("qT1", qT[1]),
                                   ("ao0", ao[0]), ("ao1", ao[1]),
                                   ("vsb", v_sb)):
                        w = src.shape[1]
                        t = dbp.tile([128, w], F32, name=f"dbg{n}", tag="dbg")
                        nc.vector.tensor_copy(t[:], src[:])
                        nc.sync.dma_start(dbg[n][:, :], t[:])

    nc.finalize()
    return nc


def _get_nc():
    if "nc" not in _CACHE:
        _CACHE["nc"] = _build()
    return _CACHE["nc"]


def _prepack(xT, ncols):
    """[D, ncols] -> partition-major [128, 8*ncols] (d-tile t at col t*ncols)."""
    return np.ascontiguousarray(
        xT.reshape(8, 128, ncols).transpose(1, 0, 2).reshape(128, 8 * ncols))


def _in_maps(Q, K, V, Wq, bq, Wk, bk, Wv, bv, Wo, bo):
    import ml_dtypes
    bf16 = ml_dtypes.bfloat16
    maps = []
    for c in range(NCORES):
        b, g = c // 4, c % 4
        sl = slice(g * OC, (g + 1) * OC)
        maps.append({
            "qt": _prepack(Q[b].T.astype(bf16), S),
            "kt": _prepack(K[b].T.astype(bf16), S),
            "vt": _prepack(V[b].T.astype(bf16), S),
            "wq": _prepack(Wq[sl, :].T.astype(bf16), OC),
            "wk": _prepack(Wk[sl, :].T.astype(bf16), OC),
            "wv": _prepack(Wv[sl, :].T.astype(bf16), OC),
            "wo": np.ascontiguousarray(Wo[:, sl].T.astype(bf16)),
            "bq": np.ascontiguousarray(
                bq[sl].astype(np.float32).reshape(2, 128).T),
            "bk": np.ascontiguousarray(
                bk[sl].astype(np.float32).reshape(2, 128).T),
        })
    return maps


def kernel(Q, K, V, Wq, bq, Wk, bk, Wv, bv, Wo, bo, validate=False, **_kw):
    from concourse.bass_utils import run_bass_kernel_spmd

    Q, K, V = (np.asarray(x, np.float32) for x in (Q, K, V))
    Wq, bq, Wk, bk, Wv, bv, Wo, bo = (
        np.asarray(x, np.float32) for x in (Wq, bq, Wk, bk, Wv, bv, Wo, bo))

    nc = _get_nc()
    res = run_bass_kernel_spmd(nc, _in_maps(Q, K, V, Wq, bq, Wk, bk, Wv, bv, Wo, bo),
                               core_ids=list(range(NCORES)))
    parts = [np.asarray(res.results[c]["out"], dtype=np.float32)
             for c in range(NCORES)]
    bo_eff = (bo.astype(np.float64) + bv.astype(np.float64) @ Wo.T.astype(np.float64)
              ).astype(np.float32)
    outs = []
    for b in range(B):
        acc = parts[4 * b].copy()
        for g in range(1, 4):
            acc += parts[4 * b + g]
        outs.append(acc + bo_eff)
    return np.stack(outs)
